# revision 27
# baseline (speedup 1.0000x reference)
"""Trainium2 Bass kernel for nn_Encoder_88983132439258 (GNN message passing).

v3 — gather rebuilt on bulk dma_gather + one-hot scatter matmuls:
  - Gather: nc.gpsimd.dma_gather on 4 SWDGE queues (ucode max), 1024
    indices per call (ring capacity), 256B elems (= one padded table row).
    Measured ~2.5 ns/edge at 4 queues vs ~8.5 at 1 (the SWDGE descriptor
    ucode parallelizes across queues; prior session's "no faster" note was
    wrong). Tables stored row-padded [N, 128] bf16 so elem stride is 256B;
    int16 gather indices split edges into two 32768-row halves.
  - Scatter: edges sorted by (dst window, half, src); per 128-edge tile a
    [128,128] one-hot (DVE tensor_scalar is_equal vs iota, exact) routes
    edge messages into the window's PSUM accumulator via one PE matmul
    (out = onehot^T @ gathered[:, :32]). Edge weights folded by one DVE
    multiply per gather call. Windows are processed sequentially — PSUM
    accumulation groups must not interleave within a bank (measured: a
    start=True wipes the whole open bank) — rotating over 4 full-bank
    PSUM tiles.
  - Natural dst order throughout (window w = dsts [128w,128w+128)): no
    degree sort, no scatter-back; layer-3 rows land in hin directly.
  - Epilogue per 4-window chunk (transpose -> ELU -> block-diag W ->
    transpose) and the FC head are carried over from the proven baseline.
"""

import numpy as np
import ml_dtypes

import concourse.bacc as bacc
import concourse.mybir as mybir
import concourse.tile as tile
import concourse.bass as bass
from concourse import bass_utils

F32 = mybir.dt.float32
BF16 = mybir.dt.bfloat16
I16 = mybir.dt.int16

N = 65536
NODES_PER = 4096
N_GRAPHS = 16
FEAT_IN = 16
HID = 32
FC_HID = 256
LATENT = 64
NC = 8
OWN = N // NC          # 8192 dsts per core
NWND = OWN // 128      # 64 windows of 128 dsts
NCHUNK = NWND // 4     # 16 epilogue chunks of 4 windows
P = 128
HALF = 32768           # int16 index limit per gather half
NQ = 4                 # SWDGE queues (ucode max)
CALL_TILES = 8         # 1024 idxs per dma_gather call (ring capacity)
_USE_COLL = True       # debug: False replaces collectives with local copies
_QOFF = 1              # gather queues: _QOFF .. _QOFF+_QMOD-1
_QMOD = 3              # (queue 0 reserved away from collectives)
_VARIANT = "full"      # debug: full | nofc | gatheronly | nogather

_prog_cache = {}


def _build_schedule(Twh):
    """Twh: [64, 2] tiles per (window, half). Tiles are laid half-major
    ([all h=0 tiles window-major][all h=1]); gather calls are uniform
    CALL_TILES chunks within each half (they may span windows — the
    per-tile one-hots route rows regardless). Matmul consumption is
    window-major, pulling from both streams."""
    ranges = {}           # (w, h) -> (t0, n)
    t = 0
    for h in range(2):
        for w in range(NWND):
            n = int(Twh[w][h])
            ranges[(w, h)] = (t, n)
            t += n
    T = t
    calls = []            # (h, t0, nt), per half, uniform chunks
    for h in range(2):
        lo = ranges[(0, h)][0]
        hi = ranges[(NWND - 1, h)][0] + ranges[(NWND - 1, h)][1]
        o = lo
        while o < hi:
            nt = min(CALL_TILES, hi - o)
            calls.append((h, o, nt))
            o += nt
    tile_first = [None] * NWND
    tile_last = [None] * NWND
    for w in range(NWND):
        t0a, n0 = ranges[(w, 0)]
        t0b, n1 = ranges[(w, 1)]
        tile_first[w] = t0a if n0 > 0 else t0b
        tile_last[w] = (t0b + n1 - 1) if n1 > 0 else (t0a + n0 - 1)
    return calls, ranges, tile_first, tile_last, T


def _build_program(Twh, reps=1):
    calls, ranges, tile_first, tile_last, T = _build_schedule(Twh)
    # interleave the two halves' call streams so consumption (window-major,
    # both halves per window) tracks the gather streams
    h0_calls = [c for c in calls if c[0] == 0]
    h1_calls = [c for c in calls if c[0] == 1]
    merged_calls = []
    for i in range(max(len(h0_calls), len(h1_calls))):
        if i < len(h0_calls):
            merged_calls.append(h0_calls[i])
        if i < len(h1_calls):
            merged_calls.append(h1_calls[i])
    nc = bacc.Bacc("TRN2", target_bir_lowering=False, debug=False,
                   num_devices=NC, num_swdge_queues=NQ)

    # ---- I/O ----
    tab1 = nc.dram_tensor("tab1", [N, P], BF16, kind="ExternalInput")
    idx_in = nc.dram_tensor("idx16", [P, T * 8], I16, kind="ExternalInput")
    dstw_in = nc.dram_tensor("dstw", [P, T], F32, kind="ExternalInput")
    wgt_in = nc.dram_tensor("wgt", [P, T], F32, kind="ExternalInput")
    iota_in = nc.dram_tensor("iotat", [P, P], BF16, kind="ExternalInput")
    wbd_in = nc.dram_tensor("wbd", [2, P, P], BF16, kind="ExternalInput")
    bst_in = nc.dram_tensor("bst", [3, P], F32, kind="ExternalInput")
    idf_in = nc.dram_tensor("identf", [P, P], F32, kind="ExternalInput")
    idb_in = nc.dram_tensor("identb", [P, P], BF16, kind="ExternalInput")
    wfc1_in = nc.dram_tensor("wfc1s", [P, 1024 * HID], BF16,
                             kind="ExternalInput")
    wfc2_in = nc.dram_tensor("wfc2s", [HID, LATENT], F32,
                             kind="ExternalInput")
    bfc1_in = nc.dram_tensor("bfc1t", [N_GRAPHS, HID], F32,
                             kind="ExternalInput")
    bfc2_in = nc.dram_tensor("bfc2t", [N_GRAPHS, LATENT], F32,
                             kind="ExternalInput")
    out = nc.dram_tensor("out", [N_GRAPHS, LATENT], F32,
                         kind="ExternalOutput")

    # ---- internal DRAM ----
    cin2 = nc.dram_tensor("cin2", [OWN, HID], BF16)
    cin3 = nc.dram_tensor("cin3", [OWN, HID], BF16)
    tabc2 = nc.dram_tensor("tabc2", [N, HID], BF16, addr_space="Shared")
    tabc3 = nc.dram_tensor("tabc3", [N, HID], BF16, addr_space="Shared")
    tab2 = nc.dram_tensor("tab2", [N, P], BF16)
    tab3 = nc.dram_tensor("tab3", [N, P], BF16)
    hin = nc.dram_tensor("hin", [2, NODES_PER * HID], BF16)
    hfull = nc.dram_tensor("hfull", [N_GRAPHS, NODES_PER * HID], BF16,
                           addr_space="Shared")
    arin = nc.dram_tensor("arin", [N_GRAPHS, LATENT], F32)
    arout = nc.dram_tensor("arout", [N_GRAPHS, LATENT], F32,
                           addr_space="Shared")

    groups = [list(range(NC))]

    with tile.TileContext(nc) as tc:
        with tc.tile_pool(name="const", bufs=1) as cst, \
             tc.tile_pool(name="gath", bufs=12) as gp, \
             tc.tile_pool(name="oh", bufs=8) as ohp, \
             tc.tile_pool(name="m2p", bufs=2) as m2p, \
             tc.tile_pool(name="small", bufs=4) as sm, \
             tc.tile_pool(name="wps", bufs=1, space="PSUM") as wpsp, \
             tc.tile_pool(name="ps1", bufs=1, space="PSUM") as ps1, \
             tc.tile_pool(name="ps2", bufs=1, space="PSUM") as ps2, \
             tc.tile_pool(name="psf", bufs=2, space="PSUM") as psf, \
             tc.tile_pool(name="psfc", bufs=1, space="PSUM") as psfc, \
             tc.tile_pool(name="fcp", bufs=3) as fcp:

            # ---- constants to SBUF ----
            idx_t = cst.tile([P, T * 8], I16, tag="idx")
            nc.sync.dma_start(out=idx_t[:], in_=idx_in[:, :])
            dstw_t = cst.tile([P, T], F32, tag="dstw")
            nc.sync.dma_start(out=dstw_t[:], in_=dstw_in[:, :])
            wgt_t = cst.tile([P, T], F32, tag="wgt")
            nc.sync.dma_start(out=wgt_t[:], in_=wgt_in[:, :])
            iota_t = cst.tile([P, P], BF16, tag="iota")
            nc.sync.dma_start(out=iota_t[:], in_=iota_in[:, :])
            wbd_t = [cst.tile([P, P], BF16, tag=f"wbd{i}", name=f"wbd_t{i}")
                     for i in range(2)]
            for i in range(2):
                nc.sync.dma_start(out=wbd_t[i][:], in_=wbd_in[i])
            bst_t = [cst.tile([P, 1], F32, tag=f"bst{i}", name=f"bst_t{i}")
                     for i in range(3)]
            for i in range(3):
                nc.sync.dma_start(out=bst_t[i][:],
                                  in_=bst_in[i].rearrange("(p o) -> p o", o=1))
            idf_t = cst.tile([P, P], F32, tag="idf")
            nc.sync.dma_start(out=idf_t[:], in_=idf_in[:, :])
            idb_t = cst.tile([P, P], BF16, tag="idb")
            nc.sync.dma_start(out=idb_t[:], in_=idb_in[:, :])
            wfc2_t = cst.tile([HID, LATENT], F32, tag="wfc2")
            nc.sync.dma_start(out=wfc2_t[:], in_=wfc2_in[:, :])
            bfc1_t = cst.tile([N_GRAPHS, HID], F32, tag="bfc1")
            nc.sync.dma_start(out=bfc1_t[:], in_=bfc1_in[:, :])
            bfc2_t = cst.tile([N_GRAPHS, LATENT], F32, tag="bfc2")
            nc.sync.dma_start(out=bfc2_t[:], in_=bfc2_in[:, :])
            wfc1_t = cst.tile([P, 1024 * HID], BF16, tag="wfc1r")
            nc.sync.dma_start(out=wfc1_t[:], in_=wfc1_in[:, :])

            import contextlib
            _loop = (tc.For_i(0, reps) if reps > 1
                     else contextlib.nullcontext())
            with _loop:
                qi = [0]

                def layer(li, tab_ap, store):
                    wtiles = [None] * 2     # psum tile per w%2
                    m2cur = [None]          # current chunk's m2 sbuf tile

                    def flush_window(w):
                        c, b = w // 4, w % 4
                        if b == 0:
                            m2cur[0] = m2p.tile([P, P], F32, tag="m2",
                                                name=f"m2_l{li}_c{c}")
                        nc.vector.tensor_copy(
                            m2cur[0][:, b * HID:(b + 1) * HID],
                            wtiles[w % 2][:, 0:HID])
                        if b == 3:
                            epilogue(li, c, m2cur[0], store)

                    def wtiles_list(w):
                        t0a, n0 = ranges[(w, 0)]
                        t0b, n1 = ranges[(w, 1)]
                        return (list(range(t0a, t0a + n0))
                                + list(range(t0b, t0b + n1)))

                    def do_window(w):
                        for t in wtiles_list(w):
                            gt, j = tile_chunk[t]
                            if t == tile_first[w]:
                                wtiles[w % 2] = wpsp.tile(
                                    [P, 512], F32, tag=f"wps{w % 2}",
                                    name=f"wps_l{li}_w{w}")
                            oh = ohp.tile([P, P], BF16, tag="oh",
                                          name=f"oh_l{li}_t{t}")
                            nc.vector.tensor_scalar(
                                out=oh[:], in0=iota_t[:],
                                scalar1=dstw_t[:, t:t + 1],
                                scalar2=wgt_t[:, t:t + 1],
                                op0=mybir.AluOpType.is_equal,
                                op1=mybir.AluOpType.mult)
                            nc.tensor.matmul(
                                out=wtiles[w % 2][:, 0:HID],
                                lhsT=oh[:],
                                rhs=gt[:, j * P:j * P + HID],
                                start=(t == tile_first[w]),
                                stop=(t == tile_last[w]))
                            if t == tile_last[w]:
                                flush_window(w)

                    # gather calls (half-streams interleaved) with
                    # window-major consumption as tiles become available;
                    # gp's buffer rotation flow-controls the Pool queue
                    tile_chunk = {}
                    wnext = 0
                    for (h, t0, nt) in merged_calls:
                        if _VARIANT == "nogather":
                            for j in range(nt):
                                tile_chunk[t0 + j] = (None, j)
                        else:
                            g = gp.tile([P, nt * P], BF16, tag="g",
                                        name=f"g_l{li}_t{t0}")
                            nc.gpsimd.dma_gather(
                                out_ap=g[:].rearrange("p (c e) -> p c e", e=P),
                                in_ap=tab_ap[h * HALF:(h + 1) * HALF, :],
                                idxs_ap=idx_t[:, t0 * 8:(t0 + nt) * 8],
                                num_idxs=nt * P,
                                num_idxs_reg=nt * P,
                                elem_size=P,
                                queue_num=(qi[0] % _QMOD) + _QOFF,
                            )
                            qi[0] += 1
                            for j in range(nt):
                                tile_chunk[t0 + j] = (g, j)
                        if _VARIANT == "gatheronly":
                            continue
                        while wnext < NWND and all(
                                t in tile_chunk for t in wtiles_list(wnext)):
                            do_window(wnext)
                            wnext += 1
                    while _VARIANT != "gatheronly" and wnext < NWND:
                        do_window(wnext)
                        wnext += 1

                def epilogue(li, c, m2, store):
                    # [128 dst, 4x32 feat] -> feature-major, ELU, (W), back
                    pt = ps1.tile([P, P], F32, tag="pt")
                    nc.tensor.transpose(out=pt[:], in_=m2[:],
                                        identity=idf_t[:])
                    rl = sm.tile([P, P], BF16, tag="rl")
                    nc.scalar.activation(rl[:], pt[:],
                                         mybir.ActivationFunctionType.Relu,
                                         bias=bst_t[li][:])
                    mn = sm.tile([P, P], F32, tag="mn")
                    nc.vector.scalar_tensor_tensor(
                        out=mn[:], in0=pt[:], scalar=bst_t[li][:],
                        in1=rl[:], op0=mybir.AluOpType.add,
                        op1=mybir.AluOpType.subtract)
                    ex = sm.tile([P, P], F32, tag="ex")
                    nc.scalar.activation(ex[:], mn[:],
                                         mybir.ActivationFunctionType.Exp)
                    if li < 2:
                        hx = sm.tile([P, P], BF16, tag="h")
                    else:
                        hx = sm.tile([P, P], F32, tag="hf")
                    nc.vector.scalar_tensor_tensor(
                        out=hx[:], in0=rl[:], scalar=-1.0, in1=ex[:],
                        op0=mybir.AluOpType.add, op1=mybir.AluOpType.add)
                    if li < 2:
                        pz = ps2.tile([P, P], F32, tag="pz")
                        nc.tensor.matmul(out=pz[:], lhsT=wbd_t[li][:],
                                         rhs=hx[:], start=True, stop=True)
                        zs = sm.tile([P, P], F32, tag="zs")
                        nc.scalar.copy(zs[:], pz[:])
                        pn = ps1.tile([P, P], F32, tag="pt")
                        nc.tensor.transpose(out=pn[:], in_=zs[:],
                                            identity=idf_t[:])
                    else:
                        pn = ps1.tile([P, P], F32, tag="pt")
                        nc.tensor.transpose(out=pn[:], in_=hx[:],
                                            identity=idf_t[:])
                    tn = sm.tile([P, P], BF16, tag="tn")
                    nc.vector.tensor_copy(tn[:], pn[:])
                    store(c, tn)

                # ---- layer 1 ----
                def store_l1(c, tn):
                    nc.sync.dma_start(
                        out=cin2.ap()[c * 512:(c + 1) * 512, :].rearrange(
                            "(b p) f -> p b f", p=P),
                        in_=tn[:].rearrange("p (b f) -> p b f", f=HID))
                layer(0, tab1.ap(), store_l1)
                if reps == 1 and _USE_COLL:
                    nc.gpsimd.collective_compute(
                        "AllGather", mybir.AluOpType.bypass,
                        replica_groups=groups,
                        ins=[cin2.ap().opt()], outs=[tabc2.ap().opt()])
                else:
                    nc.sync.dma_start(out=tabc2.ap()[:OWN, :],
                                      in_=cin2.ap())
                for xc in range(8):
                    nc.sync.dma_start(
                        out=tab2.ap()[xc * OWN:(xc + 1) * OWN, 0:HID],
                        in_=tabc2.ap()[xc * OWN:(xc + 1) * OWN, :])

                # ---- layer 2 ----
                def store_l2(c, tn):
                    nc.sync.dma_start(
                        out=cin3.ap()[c * 512:(c + 1) * 512, :].rearrange(
                            "(b p) f -> p b f", p=P),
                        in_=tn[:].rearrange("p (b f) -> p b f", f=HID))
                layer(1, tab2.ap(), store_l2)
                if reps == 1 and _USE_COLL:
                    nc.gpsimd.collective_compute(
                        "AllGather", mybir.AluOpType.bypass,
                        replica_groups=groups,
                        ins=[cin3.ap().opt()], outs=[tabc3.ap().opt()])
                else:
                    nc.sync.dma_start(out=tabc3.ap()[:OWN, :],
                                      in_=cin3.ap())
                for xc in range(8):
                    nc.sync.dma_start(
                        out=tab3.ap()[xc * OWN:(xc + 1) * OWN, 0:HID],
                        in_=tabc3.ap()[xc * OWN:(xc + 1) * OWN, :])

                # ---- layer 3: rows land in natural order ----
                hrows = hin.ap().rearrange("g (i f) -> (g i) f", f=HID)

                def store_l3(c, tn):
                    nc.sync.dma_start(
                        out=hrows[c * 512:(c + 1) * 512, :].rearrange(
                            "(b p) f -> p b f", p=P),
                        in_=tn[:].rearrange("p (b f) -> p b f", f=HID))
                layer(2, tab3.ap(), store_l3)
                if reps == 1 and _USE_COLL:
                    nc.gpsimd.collective_compute(
                        "AllGather", mybir.AluOpType.bypass,
                        replica_groups=groups,
                        ins=[hin.ap().opt()], outs=[hfull.ap().opt()])
                else:
                    nc.sync.dma_start(out=hfull.ap()[:2, :], in_=hin.ap())

                # ---- FC head; FC1 weights SBUF-resident (loaded once,
                # overlapped with the conv layers) ----
                pfc = psfc.tile([N_GRAPHS, HID], F32, tag="pfc")
                NSLAB = 32
                for slab_i in range(1024 // NSLAB):
                    slab = fcp.tile([N_GRAPHS, NSLAB * P], BF16, tag="slab")
                    nc.sync.dma_start(
                        out=slab[:],
                        in_=hfull.ap()[:, slab_i * NSLAB * P:
                                       (slab_i + 1) * NSLAB * P])
                    for pk in range(NSLAB // 8):
                        ptr = psf.tile([P, 8 * N_GRAPHS], BF16, tag="ptr")
                        for t in range(8):
                            nc.tensor.transpose(
                                out=ptr[:, t * N_GRAPHS:(t + 1) * N_GRAPHS],
                                in_=slab[:, (pk * 8 + t) * P:(pk * 8 + t + 1) * P],
                                identity=idb_t[:N_GRAPHS, :N_GRAPHS])
                        hc = fcp.tile([P, 8 * N_GRAPHS], BF16, tag="hc")
                        nc.vector.tensor_copy(hc[:], ptr[:])
                        ci0 = slab_i * NSLAB + pk * 8
                        for t in range(8):
                            ci = ci0 + t
                            nc.tensor.matmul(
                                out=pfc[:],
                                lhsT=hc[:, t * N_GRAPHS:(t + 1) * N_GRAPHS],
                                rhs=wfc1_t[:, ci * HID:(ci + 1) * HID],
                                start=(ci == 0), stop=(ci == 1023))
                u = sm.tile([N_GRAPHS, HID], F32, tag="u")
                nc.vector.tensor_tensor(out=u[:], in0=pfc[:], in1=bfc1_t[:],
                                        op=mybir.AluOpType.add)
                rlu = sm.tile([N_GRAPHS, HID], F32, tag="rlu")
                nc.scalar.activation(rlu[:], u[:],
                                     mybir.ActivationFunctionType.Relu)
                mnu = sm.tile([N_GRAPHS, HID], F32, tag="mnu")
                nc.vector.scalar_tensor_tensor(
                    out=mnu[:], in0=u[:], scalar=0.0, in1=rlu[:],
                    op0=mybir.AluOpType.add, op1=mybir.AluOpType.subtract)
                exu = sm.tile([N_GRAPHS, HID], F32, tag="exu")
                nc.scalar.activation(exu[:], mnu[:],
                                     mybir.ActivationFunctionType.Exp)
                fcm = sm.tile([N_GRAPHS, HID], F32, tag="fcm")
                nc.vector.scalar_tensor_tensor(
                    out=fcm[:], in0=rlu[:], scalar=-1.0, in1=exu[:],
                    op0=mybir.AluOpType.add, op1=mybir.AluOpType.add)
                pTf = ps1.tile([P, P], F32, tag="pt", name="pTf")
                pT = pTf[0:HID, 0:N_GRAPHS]
                nc.tensor.transpose(out=pT, in_=fcm[:],
                                    identity=idf_t[:N_GRAPHS, :N_GRAPHS])
                fcmT = sm.tile([HID, N_GRAPHS], F32, tag="fcmT")
                nc.vector.tensor_copy(fcmT[:], pT)
                pPf = ps2.tile([P, P], F32, tag="pz", name="pPf")
                pP = pPf[0:N_GRAPHS, 0:LATENT]
                nc.tensor.matmul(out=pP, lhsT=fcmT[:], rhs=wfc2_t[:],
                                 start=True, stop=True)
                part = sm.tile([N_GRAPHS, LATENT], F32, tag="part")
                nc.vector.tensor_copy(part[:], pP)
                nc.sync.dma_start(out=arin.ap(), in_=part[:])
                if reps == 1 and _USE_COLL:
                    nc.gpsimd.collective_compute(
                        "AllReduce", mybir.AluOpType.add,
                        replica_groups=groups,
                        ins=[arin.ap().opt()], outs=[arout.ap().opt()])
                else:
                    nc.sync.dma_start(out=arout.ap(), in_=arin.ap())
                res = sm.tile([N_GRAPHS, LATENT], F32, tag="res")
                nc.sync.dma_start(out=res[:], in_=arout.ap())
                fin = sm.tile([N_GRAPHS, LATENT], F32, tag="fin")
                nc.vector.tensor_tensor(out=fin[:], in0=res[:], in1=bfc2_t[:],
                                        op=mybir.AluOpType.add)
                nc.sync.dma_start(out=out.ap(), in_=fin[:])

    nc.compile()
    return nc


def _host_prep(inputs):
    x = np.asarray(inputs["x"], np.float32)
    ei = np.asarray(inputs["edge_index"])
    w = np.asarray(inputs["edge_attr"], np.float32)
    W1 = np.asarray(inputs["W1"], np.float32)
    b1 = np.asarray(inputs["b1"], np.float32)
    W2 = np.asarray(inputs["W2"], np.float32)
    b2 = np.asarray(inputs["b2"], np.float32)
    W3 = np.asarray(inputs["W3"], np.float32)
    b3 = np.asarray(inputs["b3"], np.float32)
    Wfc1 = np.asarray(inputs["Wfc1"], np.float32)
    bfc1 = np.asarray(inputs["bfc1"], np.float32)
    Wfc2 = np.asarray(inputs["Wfc2"], np.float32)
    bfc2 = np.asarray(inputs["bfc2"], np.float32)

    src = ei[0].astype(np.int64)
    dst = ei[1].astype(np.int64)

    core = dst >> 13
    wnd = (dst & 8191) >> 7
    half = (src >= HALF).astype(np.int64)

    # sort edges by (core, half, window, src) — half-major run layout
    order = np.lexsort((src, wnd, half, core))
    src_s = src[order]
    w_s = w[order]
    dst_s = dst[order]
    core_s = core[order]
    wnd_s = wnd[order]
    half_s = half[order]

    key = (core_s * NWND + wnd_s) * 2 + half_s
    cnt = np.bincount(key, minlength=NC * NWND * 2).reshape(NC, NWND, 2)
    Twh = np.maximum(np.ceil(cnt.max(axis=0) / P).astype(np.int64), 0)
    # guard: ensure every window has at least one tile
    for wv in range(NWND):
        if Twh[wv].sum() == 0:
            Twh[wv][0] = 1
    Twh_key = tuple(map(tuple, Twh.tolist()))

    # run base slot offset per (half, window) — half-major
    run_tiles = np.concatenate([Twh[:, 0], Twh[:, 1]])
    run_off = np.zeros(NWND * 2 + 1, np.int64)
    np.cumsum(run_tiles, out=run_off[1:])
    T = int(run_off[-1])

    # slot position of each edge: run_off[(h,w)]*128 + within-run index
    idx_slots = np.zeros((NC, T * P), np.int16)
    dstw_slots = np.full((NC, T * P), -1.0, np.float32)
    wgt_slots = np.zeros((NC, T * P), np.float32)

    edge_key = (half_s * NWND + wnd_s)
    ecnt = np.concatenate([cnt[:, :, 0], cnt[:, :, 1]], axis=1)
    for k in range(NC):
        msk = core_s == k
        ek = edge_key[msk]
        # edges are sorted by (wnd, half, src) within the core
        c_k = ecnt[k]
        within = np.arange(len(ek)) - np.repeat(
            np.concatenate(([0], np.cumsum(c_k)[:-1])), c_k)
        pos = run_off[ek] * P + within
        idx_slots[k, pos] = (src_s[msk] - half_s[msk] * HALF).astype(np.int16)
        dstw_slots[k, pos] = (dst_s[msk] & 127).astype(np.float32)
        wgt_slots[k, pos] = w_s[msk]

    # wrapped-16 gather index layout, replicated across the 8 gpsimd cores
    idx16 = np.zeros((NC, P, T * 8), np.int16)
    for k in range(NC):
        blk = idx_slots[k].reshape(T * 8, 16).T     # [16, T*8]
        for g in range(8):
            idx16[k, g * 16:(g + 1) * 16, :] = blk
    dstw_a = np.ascontiguousarray(
        dstw_slots.reshape(NC, T, P).transpose(0, 2, 1))
    wgt_a = np.ascontiguousarray(
        wgt_slots.reshape(NC, T, P).transpose(0, 2, 1))

    # layer-1 table, row-padded to 256B
    z1 = x @ W1
    tab1 = np.zeros((N, P), ml_dtypes.bfloat16)
    tab1[:, 0:HID] = z1.astype(ml_dtypes.bfloat16)

    iota = np.tile(np.arange(P, dtype=np.float32), (P, 1)).astype(
        ml_dtypes.bfloat16)

    def blockdiag(Wm):
        o = np.zeros((P, P), np.float32)
        for t in range(4):
            o[t * HID:(t + 1) * HID, t * HID:(t + 1) * HID] = Wm
        return o.astype(ml_dtypes.bfloat16)

    wbd = np.stack([blockdiag(W2), blockdiag(W3)])
    bst = np.stack([np.tile(b1, 4), np.tile(b2, 4),
                    np.tile(b3, 4)]).astype(np.float32)
    identf = np.eye(P, dtype=np.float32)

    in_maps = []
    for k in range(NC):
        wfc1s = np.ascontiguousarray(
            Wfc1[:, HID * k:HID * (k + 1)].reshape(1024, P, HID)
            .transpose(1, 0, 2).reshape(P, 1024 * HID)).astype(
                ml_dtypes.bfloat16)
        in_maps.append({
            "tab1": tab1,
            "idx16": idx16[k],
            "dstw": dstw_a[k],
            "wgt": wgt_a[k],
            "iotat": iota,
            "wbd": wbd,
            "bst": bst,
            "identf": identf,
            "identb": np.eye(P, dtype=ml_dtypes.bfloat16),
            "wfc1s": wfc1s,
            "wfc2s": np.ascontiguousarray(Wfc2[HID * k:HID * (k + 1), :]),
            "bfc1t": np.tile(bfc1[HID * k:HID * (k + 1)], (N_GRAPHS, 1)),
            "bfc2t": np.tile(bfc2, (N_GRAPHS, 1)),
        })
    return Twh_key, in_maps


def kernel(**inputs):
    Twh_key, in_maps = _host_prep(inputs)
    if Twh_key not in _prog_cache:
        _prog_cache[Twh_key] = _build_program(Twh_key)
    nc = _prog_cache[Twh_key]
    res = bass_utils.run_bass_kernel_spmd(nc, in_maps,
                                          core_ids=list(range(NC)))
    return np.asarray(res.results[0]["out"], np.float32)
